# revision 11
# baseline (speedup 1.0000x reference)
"""Trainium2 Bass kernel for nn_Consistent_loss_up_2 (scatter_memory).

Reference computation:
    bins = round(up*50+110) clipped to [0,255]; mask = up >= 0.0235
    scatter-max over i into up2left/up2right[k, 0, j, bin]:
        i > 128:  value (i-128)/60  -> up2right
        i <= 128: value (128-i)/60  -> up2left
    loss = mean(|up2right-right| masked) + mean(|up2left-left| masked)
    where masked = (d < 0.2) & (map != 0)

Key structure exploited:
  * only bins 111..160 (50 of 256) are reachable -> per-(k,j) tables are
    50 wide; left/right/jt/k-parity packed into 400 cols per k-pair
  * scatter values are monotone in i, so scatter-max == overwrite-scatter
    in the right stream order (left: i descending, right: i ascending);
    gpsimd local_scatter is last-write-wins (verified on HW)
  * bin rounding is done by the f32->fp16 dtype conversion in the ACT
    g-pass (values kept in [1044,1094] where fp16 ulp=1 so RNE rounds to
    exact ints, matching jnp.round) -- no 2^23 trick, no extra DVE pass
  * per-(k%2,jt,h) table column offsets {0,...,350} are added pre-transpose
    via a constant cofs tile folded into the mask subtrahend; all values
    stay < 2048 so fp16 arithmetic on them is exact
  * two batches are processed per iteration with 1024-wide ops: ACT calls
    cost ~700ns nearly independent of width, so fewer/wider wins
  * scatter calls are merged into 8 calls x 1024 idxs (call overhead
    ~200ns, marginal ~4.1ns/idx-col)
  * loss phase is chunked per pair (8 chunks, separate table tiles) and
    emitted 3 pairs late so it overlaps the scatter phase without ever
    stalling the in-order DVE queue; g for pair p+1 is emitted before
    st_p so the in-order ACT queue never serializes the pipeline

Sharding: data-parallel over batch B=128 across 8 cores (16 each);
each core returns [128,4] partial sums, host reduces.
"""

import numpy as np

from concourse import bacc, mybir, tile
from concourse.bass_utils import run_bass_kernel_spmd

B, H, W = 128, 256, 256
NCORES = 8
KPC = B // NCORES   # batches per core = 16
NBIN = 50           # reachable bins: 111..160
OFF = 1024.0        # table-value offset so empty bins auto-fail d<0.2
MASK_SUB = 4000.0   # pushes masked points' indices negative
KTBL = 200          # per-k table: [jt0L, jt0R, jt1L, jt1R] x 50 bins
NPAIR = KPC // 2    # 8 k-pairs per core
CHW = 2 * KTBL      # 400 table cols per pair
NCHUNK = 4          # loss chunks (2 pairs each)
LCW = 2 * CHW       # 800 cols per loss chunk

_cache = {}


def _build_bass():
    nc = bacc.Bacc("TRN2", target_bir_lowering=False)
    f32, i16, f16 = mybir.dt.float32, mybir.dt.int16, mybir.dt.float16
    Alu = mybir.AluOpType
    Act = mybir.ActivationFunctionType

    up_in = nc.dram_tensor("up_in", [KPC * H, W], f32, kind="ExternalInput")
    refs_in = nc.dram_tensor("refs_in", [NCHUNK * 128, LCW], f32,
                             kind="ExternalInput")
    bcol_in = nc.dram_tensor("bcol_in", [128, 2], f32, kind="ExternalInput")
    vee_in = nc.dram_tensor("vee_in", [128, 1024], i16, kind="ExternalInput")
    cofs_in = nc.dram_tensor("cofs_in", [128, 1024], f16, kind="ExternalInput")
    ident_in = nc.dram_tensor("ident_in", [128, 128], f16, kind="ExternalInput")
    antid_in = nc.dram_tensor("antid_in", [128, 128], f16, kind="ExternalInput")
    out = nc.dram_tensor("out", [128, 2 * NCHUNK], f32, kind="ExternalOutput")

    with tile.TileContext(nc) as tc:
        with (
            tc.tile_pool(name="const", bufs=1) as constp,
            tc.tile_pool(name="tblp", bufs=1) as tblp,
            tc.tile_pool(name="refp", bufs=1) as refp,
            tc.tile_pool(name="work", bufs=4) as workp,
            tc.tile_pool(name="psum", bufs=4, space="PSUM") as psump,
            tc.tile_pool(name="loss", bufs=2) as lossp,
        ):
            # prefetch the first up pairs before anything else (sync queue)
            uts = {}

            def fetch_ut(p, split=False):
                uts[p] = workp.tile([128, 4, W], f32, tag="ut", name=f"ut{p}")
                if split:
                    # fan the first batch-pair across 4 DMA queues so the
                    # pipeline head is not gated on one 1MB transfer
                    qs = [nc.sync, nc.scalar, nc.gpsimd, nc.sync]
                    base = p * 2 * H
                    for q in range(4):
                        srcq = up_in[base + q * 128:base + (q + 1) * 128, :]
                        qs[q].dma_start(
                            uts[p][:, q, :], srcq.rearrange("p w -> p w")
                        )
                else:
                    src = up_in[p * 2 * H:(p + 1) * 2 * H, :].rearrange(
                        "(q p) w -> p q w", q=4
                    )
                    nc.sync.dma_start(uts[p][:], src)

            fetch_ut(0, split=True)
            for p in range(1, 3):
                fetch_ut(p)

            # small consts + vee/cofs on the gpsimd queue (idle until the
            # first scatter), perm matrices on sync (needed by PE early)
            bcol = constp.tile([128, 2], f32)
            nc.gpsimd.dma_start(bcol[:], bcol_in[:])
            vee = constp.tile([128, 1024], i16)
            nc.gpsimd.dma_start(vee[:], vee_in[:])
            cofs = constp.tile([128, 1024], f16)
            nc.gpsimd.dma_start(cofs[:], cofs_in[:])
            ident = constp.tile([128, 128], f16)
            nc.sync.dma_start(ident[:], ident_in[:])
            antid = constp.tile([128, 128], f16)
            nc.sync.dma_start(antid[:], antid_in[:])

            refs_sb = [None] * NCHUNK

            def fetch_refs(c):
                r = refp.tile([128, LCW], f32, tag=f"refs{c}",
                              name=f"refs{c}")
                nc.sync.dma_start(r[:], refs_in[c * 128:(c + 1) * 128, :])
                refs_sb[c] = r

            fetch_refs(0)

            tbl = [
                tblp.tile([128, LCW], i16, tag=f"tbl{c}", name=f"tbl{c}")
                for c in range(NCHUNK)
            ]
            rparts = [
                lossp.tile([128, 1], f32, tag=f"rp{c}", name=f"rp{c}")
                for c in range(NCHUNK)
            ]
            gparts = [
                lossp.tile([128, 1], f32, tag=f"gp{c}", name=f"gp{c}")
                for c in range(NCHUNK)
            ]

            # g for pair 0 ahead of the loop so the ACT queue never makes
            # g_{p+1} wait behind st_p
            gs = {}

            def emit_g(p):
                gs[p] = workp.tile([128, 1024], f16, tag="g", name=f"g{p}")
                utm = uts[p][:].rearrange("p q w -> p (q w)")
                nc.scalar.activation(gs[p][:], utm, Act.Copy, bias=1044.0,
                                     scale=50.0)

            emit_g(0)

            def emit_loss(c):
                # loss for chunk c (pairs 2c, 2c+1), emitted well after its
                # scatters so the DVE queue never stalls on the semaphore.
                # DVE does only the subtract; the windowed sum
                # S = sum(a*[a<11.5]) comes from two ACT accumulations:
                #   R = sum relu(12 - a)   (= 12*N - S over a < 12)
                #   G = sum sign(a - 12)   (= M - 2N; a == 12 has measure
                #       zero since a = |60r - 60v| with continuous r)
                # ACT is immune to the gpsimd SBUF-port contention that
                # cripples DVE during local_scatter.
                e2 = lossp.tile([128, LCW], f32, tag="e2", name=f"e2_{c}")
                nc.vector.tensor_tensor(
                    out=e2[:], in0=refs_sb[c][:], in1=tbl[c][:],
                    op=Alu.subtract,
                )
                a = lossp.tile([128, LCW], f32, tag="a", name=f"a{c}")
                nc.scalar.activation(a[:], e2[:], Act.Abs)
                r1 = lossp.tile([128, LCW], f32, tag="r1", name=f"r1_{c}")
                nc.scalar.activation(
                    r1[:], a[:], Act.Relu, bias=bcol[:, 0:1], scale=-1.0,
                    accum_out=rparts[c][:],
                )
                sg = lossp.tile([128, LCW], f32, tag="sg", name=f"sg{c}")
                nc.scalar.activation(
                    sg[:], a[:], Act.Sign, bias=bcol[:, 1:2], scale=1.0,
                    accum_out=gparts[c][:],
                )
                nc.scalar.dma_start(out[:, 2 * c:2 * c + 1], rparts[c][:])
                nc.scalar.dma_start(out[:, 2 * c + 1:2 * c + 2], gparts[c][:])

            for p in range(NPAIR):
                if p + 3 < NPAIR:
                    fetch_ut(p + 3)
                if p % 2 == 0 and p // 2 + 1 < NCHUNK:
                    fetch_refs(p // 2 + 1)
                if p + 1 < NPAIR:
                    emit_g(p + 1)

                ut = uts.pop(p)
                utm = ut[:].rearrange("p q w -> p (q w)")
                g = gs.pop(p)

                # mkx = (u < 0.0235)*4000 - cofs
                mk = workp.tile([128, 1024], f16, tag="mk")
                nc.vector.tensor_scalar(
                    mk[:], utm, 0.0235, MASK_SUB, op0=Alu.is_lt, op1=Alu.mult
                )
                mkx = workp.tile([128, 1024], f16, tag="mkx")
                nc.vector.tensor_tensor(
                    out=mkx[:], in0=mk[:], in1=cofs[:], op=Alu.subtract
                )
                # ix = g - mkx: bin+1044+cofs (valid) | very negative (masked)
                ix = workp.tile([128, 1024], f16, tag="ix")
                nc.vector.tensor_tensor(
                    out=ix[:], in0=g[:], in1=mkx[:], op=Alu.subtract
                )

                # transpose to partition=j; 8 blocks (k-parity, jt, h)
                ps = psump.tile([128, 1024], f16, tag="ps", space="PSUM")
                for kk in range(2):
                    o = kk * 512
                    nc.tensor.transpose(ps[:, o:o + 128],
                                        ix[:, o:o + 128], antid[:])
                    nc.tensor.transpose(ps[:, o + 128:o + 256],
                                        ix[:, o + 256:o + 384], ident[:])
                    nc.tensor.transpose(ps[:, o + 256:o + 384],
                                        ix[:, o + 128:o + 256], antid[:])
                    nc.tensor.transpose(ps[:, o + 384:o + 512],
                                        ix[:, o + 384:o + 512], ident[:])

                # idx = ps - 1045 in [0,400) valid, negative = skip
                st = workp.tile([128, 1024], i16, tag="st")
                nc.scalar.activation(st[:], ps[:], Act.Copy, bias=-1045.0,
                                     scale=1.0)

                c, half = p // 2, p % 2
                nc.gpsimd.local_scatter(
                    tbl[c][:, half * CHW:(half + 1) * CHW],
                    vee[:],
                    st[:],
                    channels=128,
                    num_elems=2 * KTBL,
                    num_idxs=1024,
                )
                if p >= 4 and p % 2 == 0:
                    emit_loss((p - 4) // 2)

            for c in range(NCHUNK - 2, NCHUNK):
                emit_loss(c)

    nc.compile()
    return nc


def _host_constants():
    # scatter data stream values per 256-block: [left: n+1 | right: n-128],
    # +OFF; garbage 25000 at the right-stream head (i==128, value 0 in the
    # reference -> must never produce a live table entry by itself)
    n = np.arange(256)
    blk = np.where(n < 128, n + 1, n - 128).astype(np.int64) + int(OFF)
    blk[128] = 25000
    vee = np.tile(blk, 4).astype(np.int16)
    vee = np.ascontiguousarray(np.broadcast_to(vee, (128, 1024)))

    # cofs[col]: block col offset {0,50,100,150} + 200*(k%2);
    # pre-transpose layout: col = (k%2)*512 + h*256 + j
    col = np.arange(1024)
    kk = col // 512
    h = (col % 512) // 256
    j = col % 256
    jt = (j // 128).astype(np.int64)
    cofs = (200 * kk + 100 * jt + 50 * h).astype(np.float32)
    cofs = np.ascontiguousarray(
        np.broadcast_to(cofs.astype(np.float16), (128, 1024))
    )

    ident = np.eye(128, dtype=np.float32).astype(np.float16)
    antid = np.ascontiguousarray(ident[::-1, :])
    return vee, cofs, ident, antid


def _prep_refs(left, right):
    """[CHUNKS*128, CHW] per core; table col (within core) =
    k*200 + block*50 + (bin-111), block in [jt0L, jt0R, jt1L, jt1R],
    channel = j mod 128; values pre-scaled: 60*ref + OFF."""
    lft = left[:, 0, :, 111:161]    # [B, W, 50]
    rgt = right[:, 0, :, 111:161]
    refs = np.empty((NCORES, KPC, 4, 128, NBIN), np.float32)
    for core in range(NCORES):
        for k in range(KPC):
            kg = core * KPC + k
            refs[core, k, 0] = lft[kg, 0:128, :]
            refs[core, k, 1] = rgt[kg, 0:128, :]
            refs[core, k, 2] = lft[kg, 128:256, :]
            refs[core, k, 3] = rgt[kg, 128:256, :]
    refs = refs * 60.0 + np.float32(OFF)
    # -> [core, chunk, channel, (k_in_chunk, block, bin)]
    refs = refs.reshape(NCORES, NCHUNK, 4, 4, 128, NBIN)
    refs = refs.transpose(0, 1, 4, 2, 3, 5)
    return np.ascontiguousarray(
        refs.reshape(NCORES, NCHUNK * 128, LCW).astype(np.float32)
    )


def make_in_maps(up, left, right):
    up = np.asarray(up, np.float32)
    left = np.asarray(left, np.float32)
    right = np.asarray(right, np.float32)
    vee, cofs, ident, antid = _host_constants()
    bcol = np.ascontiguousarray(np.broadcast_to(
        np.array([12.0, -12.0], np.float32), (128, 2)))
    refs = _prep_refs(left, right)
    in_maps = []
    for c in range(NCORES):
        upc = np.ascontiguousarray(
            up[c * KPC:(c + 1) * KPC, 0].reshape(KPC * H, W)
        )
        in_maps.append({
            "up_in": upc,
            "refs_in": refs[c],
            "bcol_in": bcol,
            "vee_in": vee,
            "cofs_in": cofs,
            "ident_in": ident,
            "antid_in": antid,
        })
    return in_maps


def get_nc():
    if "nc" not in _cache:
        _cache["nc"] = _build_bass()
    return _cache["nc"]


def reduce_results(results):
    # out cols: [R_0, G_0, R_1, G_1, ...] per chunk;
    # S = sum a*[a<12] = 12*N - R with N = (M - G)/2, M = LCW per
    # partition per chunk
    total = 0.0
    for r in results:
        o = np.asarray(r["out"]).astype(np.float64)
        R = o[:, 0::2].sum()
        G = o[:, 1::2].sum()
        M = 128.0 * LCW * NCHUNK
        N = (M - G) / 2.0
        total += 12.0 * N - R
    return np.float32(total / (60.0 * B * W * W))


def kernel(up, left, right):
    nc = get_nc()
    in_maps = make_in_maps(up, left, right)
    res = run_bass_kernel_spmd(nc, in_maps, core_ids=list(range(NCORES)))
    return reduce_results(res.results)


# revision 15
# speedup vs baseline: 1.0359x; 1.0359x over previous
"""Trainium2 Bass kernel for nn_Consistent_loss_up_2 (scatter_memory).

Reference computation:
    bins = round(up*50+110) clipped to [0,255]; mask = up >= 0.0235
    scatter-max over i into up2left/up2right[k, 0, j, bin]:
        i > 128:  value (i-128)/60  -> up2right
        i <= 128: value (128-i)/60  -> up2left
    loss = mean(|up2right-right| masked) + mean(|up2left-left| masked)
    where masked = (d < 0.2) & (map != 0)

Key structure exploited:
  * only bins 111..160 (50 of 256) are reachable -> per-(k,j) tables are
    50 wide; left/right/jt/k-parity packed into 400 cols per k-pair
  * scatter values are monotone in i, so scatter-max == overwrite-scatter
    in the right stream order (left: i descending, right: i ascending);
    gpsimd local_scatter is last-write-wins (verified on HW)
  * bin rounding is done by the f32->fp16 dtype conversion in the ACT
    g-pass (values kept in [1044,1094] where fp16 ulp=1 so RNE rounds to
    exact ints, matching jnp.round) -- no 2^23 trick, no extra DVE pass
  * per-(k%2,jt,h) table column offsets {0,...,350} are added pre-transpose
    via a constant cofs tile folded into the mask subtrahend; all values
    stay < 2048 so fp16 arithmetic on them is exact
  * two batches are processed per iteration with 1024-wide ops: ACT calls
    cost ~700ns nearly independent of width, so fewer/wider wins
  * scatter calls are merged into 8 calls x 1024 idxs (call overhead
    ~200ns, marginal ~4.1ns/idx-col)
  * loss phase is chunked per pair (8 chunks, separate table tiles) and
    emitted 3 pairs late so it overlaps the scatter phase without ever
    stalling the in-order DVE queue; g for pair p+1 is emitted before
    st_p so the in-order ACT queue never serializes the pipeline

Sharding: data-parallel over batch B=128 across 8 cores (16 each);
each core returns [128,4] partial sums, host reduces.
"""

import numpy as np

from concourse import bacc, mybir, tile
from concourse.bass_utils import run_bass_kernel_spmd

B, H, W = 128, 256, 256
NCORES = 8
KPC = B // NCORES   # batches per core = 16
NBIN = 50           # reachable bins: 111..160
OFF = 1024.0        # table-value offset so empty bins auto-fail d<0.2
MASK_SUB = 4000.0   # pushes masked points' indices negative
KTBL = 200          # per-k table: [jt0L, jt0R, jt1L, jt1R] x 50 bins
NPAIR = KPC // 2    # 8 k-pairs per core
CHW = 2 * KTBL      # 400 table cols per pair
NCHUNK = 4          # loss chunks (2 pairs each)
LCW = 2 * CHW       # 800 cols per loss chunk

_cache = {}


def _build_bass():
    nc = bacc.Bacc("TRN2", target_bir_lowering=False)
    f32, i16, f16 = mybir.dt.float32, mybir.dt.int16, mybir.dt.float16
    Alu = mybir.AluOpType
    Act = mybir.ActivationFunctionType

    up_in = nc.dram_tensor("up_in", [KPC * H, W], f32, kind="ExternalInput")
    refs_in = nc.dram_tensor("refs_in", [NCHUNK * 128, LCW], f32,
                             kind="ExternalInput")
    bcol_in = nc.dram_tensor("bcol_in", [128, 2], f32, kind="ExternalInput")
    vee_in = nc.dram_tensor("vee_in", [128, 1024], i16, kind="ExternalInput")
    cofs_in = nc.dram_tensor("cofs_in", [128, 1024], f16, kind="ExternalInput")
    ident_in = nc.dram_tensor("ident_in", [128, 128], f16, kind="ExternalInput")
    antid_in = nc.dram_tensor("antid_in", [128, 128], f16, kind="ExternalInput")
    out = nc.dram_tensor("out", [128, 2 * NCHUNK], f32, kind="ExternalOutput")

    with tile.TileContext(nc) as tc:
        with (
            tc.tile_pool(name="const", bufs=1) as constp,
            tc.tile_pool(name="tblp", bufs=1) as tblp,
            tc.tile_pool(name="refp", bufs=1) as refp,
            tc.tile_pool(name="work", bufs=4) as workp,
            tc.tile_pool(name="psum", bufs=4, space="PSUM") as psump,
            tc.tile_pool(name="loss", bufs=2) as lossp,
        ):
            # prefetch the first up pairs before anything else (sync queue)
            uts = {}

            def fetch_ut(p, split=False):
                uts[p] = workp.tile([128, 4 * W], f32, tag="ut",
                                    name=f"ut{p}")
                if split:
                    # fan the first batch-pair across 3 DMA queues so the
                    # pipeline head is not gated on one 1MB transfer
                    qs = [nc.sync, nc.scalar, nc.gpsimd, nc.sync]
                    base = p * 2 * H
                    for q in range(4):
                        srcq = up_in[base + q * 128:base + (q + 1) * 128, :]
                        qs[q].dma_start(uts[p][:, q * W:(q + 1) * W], srcq)
                else:
                    src = up_in[p * 2 * H:(p + 1) * 2 * H, :].rearrange(
                        "(q p) w -> p q w", q=4
                    )
                    dst = uts[p][:].rearrange("p (q w) -> p q w", q=4)
                    nc.sync.dma_start(dst, src)

            # ncofs (the mask/offset const) gates the first DVE STT: load
            # it first on the fast sync queue, then the first ut pair, then
            # everything else
            ncofs = constp.tile([128, 1024], f16)
            nc.sync.dma_start(ncofs[:], cofs_in[:])
            fetch_ut(0, split=True)
            bcol = constp.tile([128, 2], f32)
            nc.gpsimd.dma_start(bcol[:], bcol_in[:])
            vee = constp.tile([128, 1024], i16)
            nc.sync.dma_start(vee[:], vee_in[:])
            ident = constp.tile([128, 128], f16)
            nc.sync.dma_start(ident[:], ident_in[:])
            antid = constp.tile([128, 128], f16)
            nc.sync.dma_start(antid[:], antid_in[:])
            for p in range(1, 3):
                fetch_ut(p)

            refs_sb = [None] * NCHUNK

            def fetch_refs(c):
                r = refp.tile([128, LCW], f32, tag=f"refs{c}",
                              name=f"refs{c}")
                nc.sync.dma_start(r[:], refs_in[c * 128:(c + 1) * 128, :])
                refs_sb[c] = r

            fetch_refs(0)

            tbl = [
                tblp.tile([128, LCW], i16, tag=f"tbl{c}", name=f"tbl{c}")
                for c in range(NCHUNK)
            ]
            rparts = [
                lossp.tile([128, 1], f32, tag=f"rp{c}", name=f"rp{c}")
                for c in range(NCHUNK)
            ]
            gparts = [
                lossp.tile([128, 1], f32, tag=f"gp{c}", name=f"gp{c}")
                for c in range(NCHUNK)
            ]

            # g for pair 0 ahead of the loop so the ACT queue never makes
            # g_{p+1} wait behind st_p
            gs = {}

            def emit_g(p):
                gs[p] = workp.tile([128, 1024], f16, tag="g", name=f"g{p}")
                nc.scalar.activation(gs[p][:], uts[p][:], Act.Copy,
                                     bias=1044.0, scale=50.0)

            emit_g(0)

            def emit_loss(c):
                # loss for chunk c (pairs 2c, 2c+1), emitted well after its
                # scatters so the DVE queue never stalls on the semaphore.
                # DVE does only the subtract; the windowed sum
                # S = sum(a*[a<11.5]) comes from two ACT accumulations:
                #   R = sum relu(12 - a)   (= 12*N - S over a < 12)
                #   G = sum sign(a - 12)   (= M - 2N; a == 12 has measure
                #       zero since a = |60r - 60v| with continuous r)
                # ACT is immune to the gpsimd SBUF-port contention that
                # cripples DVE during local_scatter.
                e2 = lossp.tile([128, LCW], f32, tag="e2", name=f"e2_{c}")
                nc.vector.tensor_tensor(
                    out=e2[:], in0=refs_sb[c][:], in1=tbl[c][:],
                    op=Alu.subtract,
                )
                a = lossp.tile([128, LCW], f32, tag="a", name=f"a{c}")
                nc.scalar.activation(a[:], e2[:], Act.Abs)
                r1 = lossp.tile([128, LCW], f32, tag="r1", name=f"r1_{c}")
                nc.scalar.activation(
                    r1[:], a[:], Act.Relu, bias=bcol[:, 0:1], scale=-1.0,
                    accum_out=rparts[c][:],
                )
                sg = lossp.tile([128, LCW], f32, tag="sg", name=f"sg{c}")
                nc.scalar.activation(
                    sg[:], a[:], Act.Sign, bias=bcol[:, 1:2], scale=1.0,
                    accum_out=gparts[c][:],
                )
                nc.scalar.dma_start(out[:, 2 * c:2 * c + 1], rparts[c][:])
                nc.scalar.dma_start(out[:, 2 * c + 1:2 * c + 2], gparts[c][:])

            for p in range(NPAIR):
                if p + 3 < NPAIR:
                    fetch_ut(p + 3)
                if p % 2 == 0 and p // 2 + 1 < NCHUNK:
                    fetch_refs(p // 2 + 1)
                if p + 1 < NPAIR:
                    emit_g(p + 1)

                ut = uts.pop(p)
                g = gs.pop(p)

                # mk2 = (u < 0.0235) - cofs/4000; ix = -4000*mk2 + g
                #     = g - 4000*mask + cofs (to within 0.25, rounded away
                #       by the f16 grid at |ix| in [1024,2048))
                mk2 = workp.tile([128, 1024], f16, tag="mk2")
                nc.vector.scalar_tensor_tensor(
                    mk2[:], ut[:], 0.0235, ncofs[:],
                    op0=Alu.is_lt, op1=Alu.add,
                )
                ix = workp.tile([128, 1024], f16, tag="ix")
                nc.vector.scalar_tensor_tensor(
                    ix[:], mk2[:], -MASK_SUB, g[:],
                    op0=Alu.mult, op1=Alu.add,
                )

                # transpose to partition=j; 8 blocks (k-parity, jt, h)
                ps = psump.tile([128, 1024], f16, tag="ps", space="PSUM")
                for kk in range(2):
                    o = kk * 512
                    nc.tensor.transpose(ps[:, o:o + 128],
                                        ix[:, o:o + 128], antid[:])
                    nc.tensor.transpose(ps[:, o + 128:o + 256],
                                        ix[:, o + 256:o + 384], ident[:])
                    nc.tensor.transpose(ps[:, o + 256:o + 384],
                                        ix[:, o + 128:o + 256], antid[:])
                    nc.tensor.transpose(ps[:, o + 384:o + 512],
                                        ix[:, o + 384:o + 512], ident[:])

                # idx = ps - 1045 in [0,400) valid, negative = skip
                st = workp.tile([128, 1024], i16, tag="st")
                nc.scalar.activation(st[:], ps[:], Act.Copy, bias=-1045.0,
                                     scale=1.0)

                c, half = p // 2, p % 2
                nc.gpsimd.local_scatter(
                    tbl[c][:, half * CHW:(half + 1) * CHW],
                    vee[:],
                    st[:],
                    channels=128,
                    num_elems=2 * KTBL,
                    num_idxs=1024,
                )
                if p >= 4 and p % 2 == 0:
                    emit_loss((p - 4) // 2)

            for c in range(NCHUNK - 2, NCHUNK):
                emit_loss(c)

    nc.compile()
    return nc


def _host_constants():
    # scatter data stream values per 256-block: [left: n+1 | right: n-128],
    # +OFF; garbage 25000 at the right-stream head (i==128, value 0 in the
    # reference -> must never produce a live table entry by itself)
    n = np.arange(256)
    blk = np.where(n < 128, n + 1, n - 128).astype(np.int64) + int(OFF)
    blk[128] = 25000
    vee = np.tile(blk, 4).astype(np.int16)
    vee = np.ascontiguousarray(np.broadcast_to(vee, (128, 1024)))

    # cofs[col]: block col offset {0,50,100,150} + 200*(k%2);
    # pre-transpose layout: col = (k%2)*512 + h*256 + j
    col = np.arange(1024)
    kk = col // 512
    h = (col % 512) // 256
    j = col % 256
    jt = (j // 128).astype(np.int64)
    cofs = (200 * kk + 100 * jt + 50 * h).astype(np.float32)
    cofs = np.ascontiguousarray(
        np.broadcast_to((-cofs / MASK_SUB).astype(np.float16), (128, 1024))
    )

    ident = np.eye(128, dtype=np.float32).astype(np.float16)
    antid = np.ascontiguousarray(ident[::-1, :])
    return vee, cofs, ident, antid


def _prep_refs(left, right):
    """[CHUNKS*128, CHW] per core; table col (within core) =
    k*200 + block*50 + (bin-111), block in [jt0L, jt0R, jt1L, jt1R],
    channel = j mod 128; values pre-scaled: 60*ref + OFF."""
    lft = left[:, 0, :, 111:161]    # [B, W, 50]
    rgt = right[:, 0, :, 111:161]
    refs = np.empty((NCORES, KPC, 4, 128, NBIN), np.float32)
    for core in range(NCORES):
        for k in range(KPC):
            kg = core * KPC + k
            refs[core, k, 0] = lft[kg, 0:128, :]
            refs[core, k, 1] = rgt[kg, 0:128, :]
            refs[core, k, 2] = lft[kg, 128:256, :]
            refs[core, k, 3] = rgt[kg, 128:256, :]
    refs = refs * 60.0 + np.float32(OFF)
    # -> [core, chunk, channel, (k_in_chunk, block, bin)]
    refs = refs.reshape(NCORES, NCHUNK, 4, 4, 128, NBIN)
    refs = refs.transpose(0, 1, 4, 2, 3, 5)
    return np.ascontiguousarray(
        refs.reshape(NCORES, NCHUNK * 128, LCW).astype(np.float32)
    )


def make_in_maps(up, left, right):
    up = np.asarray(up, np.float32)
    left = np.asarray(left, np.float32)
    right = np.asarray(right, np.float32)
    vee, cofs, ident, antid = _host_constants()
    bcol = np.ascontiguousarray(np.broadcast_to(
        np.array([12.0, -12.0], np.float32), (128, 2)))
    refs = _prep_refs(left, right)
    in_maps = []
    for c in range(NCORES):
        upc = np.ascontiguousarray(
            up[c * KPC:(c + 1) * KPC, 0].reshape(KPC * H, W)
        )
        in_maps.append({
            "up_in": upc,
            "refs_in": refs[c],
            "bcol_in": bcol,
            "vee_in": vee,
            "cofs_in": cofs,
            "ident_in": ident,
            "antid_in": antid,
        })
    return in_maps


def get_nc():
    if "nc" not in _cache:
        _cache["nc"] = _build_bass()
    return _cache["nc"]


def reduce_results(results):
    # out cols: [R_0, G_0, R_1, G_1, ...] per chunk;
    # S = sum a*[a<12] = 12*N - R with N = (M - G)/2, M = LCW per
    # partition per chunk
    total = 0.0
    for r in results:
        o = np.asarray(r["out"]).astype(np.float64)
        R = o[:, 0::2].sum()
        G = o[:, 1::2].sum()
        M = 128.0 * LCW * NCHUNK
        N = (M - G) / 2.0
        total += 12.0 * N - R
    return np.float32(total / (60.0 * B * W * W))


def kernel(up, left, right):
    nc = get_nc()
    in_maps = make_in_maps(up, left, right)
    res = run_bass_kernel_spmd(nc, in_maps, core_ids=list(range(NCORES)))
    return reduce_results(res.results)


# revision 18
# speedup vs baseline: 1.0375x; 1.0016x over previous
"""Trainium2 Bass kernel for nn_Consistent_loss_up_2 (scatter_memory).

Reference computation:
    bins = round(up*50+110) clipped to [0,255]; mask = up >= 0.0235
    scatter-max over i into up2left/up2right[k, 0, j, bin]:
        i > 128:  value (i-128)/60  -> up2right
        i <= 128: value (128-i)/60  -> up2left
    loss = mean(|up2right-right| masked) + mean(|up2left-left| masked)
    where masked = (d < 0.2) & (map != 0)

Key structure exploited:
  * only bins 111..160 (50 of 256) are reachable -> per-(k,j) tables are
    50 wide; left/right/jt/k-parity packed into 400 cols per k-pair
  * scatter values are monotone in i, so scatter-max == overwrite-scatter
    in the right stream order (left: i descending, right: i ascending);
    gpsimd local_scatter is last-write-wins (verified on HW)
  * bin rounding is done by the f32->fp16 dtype conversion in the ACT
    g-pass (values kept in [1044,1094] where fp16 ulp=1 so RNE rounds to
    exact ints, matching jnp.round) -- no 2^23 trick, no extra DVE pass
  * per-(k%2,jt,h) table column offsets {0,...,350} are added pre-transpose
    via a constant cofs tile folded into the mask subtrahend; all values
    stay < 2048 so fp16 arithmetic on them is exact
  * two batches are processed per iteration with 1024-wide ops: ACT calls
    cost ~700ns nearly independent of width, so fewer/wider wins
  * scatter calls are merged into 8 calls x 1024 idxs (call overhead
    ~200ns, marginal ~4.1ns/idx-col)
  * loss phase is chunked per pair (8 chunks, separate table tiles) and
    emitted 3 pairs late so it overlaps the scatter phase without ever
    stalling the in-order DVE queue; g for pair p+1 is emitted before
    st_p so the in-order ACT queue never serializes the pipeline

Sharding: data-parallel over batch B=128 across 8 cores (16 each);
each core returns [128,4] partial sums, host reduces.
"""

import numpy as np

from concourse import bacc, mybir, tile
from concourse.bass_utils import run_bass_kernel_spmd

B, H, W = 128, 256, 256
NCORES = 8
KPC = B // NCORES   # batches per core = 16
NBIN = 50           # reachable bins: 111..160
OFF = 1024.0        # table-value offset so empty bins auto-fail d<0.2
MASK_SUB = 4000.0   # pushes masked points' indices negative
KTBL = 200          # per-k table: [jt0L, jt0R, jt1L, jt1R] x 50 bins
NPAIR = KPC // 2    # 8 k-pairs per core
CHW = 2 * KTBL      # 400 table cols per pair
NCHUNK = 4          # loss chunks (2 pairs each)
LCW = 2 * CHW       # 800 cols per loss chunk

_cache = {}


def _build_bass():
    nc = bacc.Bacc("TRN2", target_bir_lowering=False)
    f32, i16, f16 = mybir.dt.float32, mybir.dt.int16, mybir.dt.float16
    Alu = mybir.AluOpType
    Act = mybir.ActivationFunctionType

    up_in = nc.dram_tensor("up_in", [KPC * H, W], f32, kind="ExternalInput")
    refs_in = nc.dram_tensor("refs_in", [NCHUNK * 128, LCW], f32,
                             kind="ExternalInput")
    bcol_in = nc.dram_tensor("bcol_in", [128, 2], f32, kind="ExternalInput")
    vee_in = nc.dram_tensor("vee_in", [128, 1024], i16, kind="ExternalInput")
    cofs_in = nc.dram_tensor("cofs_in", [128, 1024], f16, kind="ExternalInput")
    ident_in = nc.dram_tensor("ident_in", [128, 128], f16, kind="ExternalInput")
    antid_in = nc.dram_tensor("antid_in", [128, 128], f16, kind="ExternalInput")
    out = nc.dram_tensor("out", [128, 2 * NCHUNK], f32, kind="ExternalOutput")

    with tile.TileContext(nc) as tc:
        with (
            tc.tile_pool(name="const", bufs=1) as constp,
            tc.tile_pool(name="tblp", bufs=1) as tblp,
            tc.tile_pool(name="refp", bufs=1) as refp,
            tc.tile_pool(name="work", bufs=6) as workp,
            tc.tile_pool(name="psum", bufs=6, space="PSUM") as psump,
            tc.tile_pool(name="loss", bufs=3) as lossp,
        ):
            # prefetch the first up pairs before anything else (sync queue)
            uts = {}

            def fetch_ut(p, split=False):
                uts[p] = workp.tile([128, 4 * W], f32, tag="ut",
                                    name=f"ut{p}")
                if split:
                    # fan the first batch-pair across 3 DMA queues so the
                    # pipeline head is not gated on one 1MB transfer
                    qs = [nc.sync, nc.scalar, nc.gpsimd, nc.sync]
                    base = p * 2 * H
                    for q in range(4):
                        srcq = up_in[base + q * 128:base + (q + 1) * 128, :]
                        qs[q].dma_start(uts[p][:, q * W:(q + 1) * W], srcq)
                else:
                    src = up_in[p * 2 * H:(p + 1) * 2 * H, :].rearrange(
                        "(q p) w -> p q w", q=4
                    )
                    dst = uts[p][:].rearrange("p (q w) -> p q w", q=4)
                    nc.sync.dma_start(dst, src)

            # cofs (the mask/offset const) gates the first DVE combine:
            # load it first on the fast sync queue, then the first ut pair,
            # then everything else
            cofs = constp.tile([128, 1024], f16)
            nc.sync.dma_start(cofs[:], cofs_in[:])
            fetch_ut(0, split=True)
            bcol = constp.tile([128, 2], f32)
            nc.gpsimd.dma_start(bcol[:], bcol_in[:])
            vee = constp.tile([128, 1024], i16)
            nc.sync.dma_start(vee[:], vee_in[:])
            ident = constp.tile([128, 128], f16)
            nc.sync.dma_start(ident[:], ident_in[:])
            antid = constp.tile([128, 128], f16)
            nc.sync.dma_start(antid[:], antid_in[:])
            for p in range(1, 3):
                fetch_ut(p)

            refs_sb = [None] * NCHUNK

            def fetch_refs(c):
                r = refp.tile([128, LCW], f32, tag=f"refs{c}",
                              name=f"refs{c}")
                nc.sync.dma_start(r[:], refs_in[c * 128:(c + 1) * 128, :])
                refs_sb[c] = r

            fetch_refs(0)

            tbl = [
                tblp.tile([128, LCW], i16, tag=f"tbl{c}", name=f"tbl{c}")
                for c in range(NCHUNK)
            ]
            rparts = [
                lossp.tile([128, 1], f32, tag=f"rp{c}", name=f"rp{c}")
                for c in range(NCHUNK)
            ]
            gparts = [
                lossp.tile([128, 1], f32, tag=f"gp{c}", name=f"gp{c}")
                for c in range(NCHUNK)
            ]

            # g emitted one pair ahead so the ACT queue never makes
            # g_{p+1} wait behind st_p
            gs = {}

            def emit_g(p):
                gs[p] = workp.tile([128, 1024], f16, tag="g", name=f"g{p}")
                nc.scalar.activation(gs[p][:], uts[p][:], Act.Copy,
                                     bias=1044.0, scale=50.0)

            def emit_loss(c):
                # loss for chunk c (pairs 2c, 2c+1), emitted well after its
                # scatters so the DVE queue never stalls on the semaphore.
                # DVE does only the subtract; the windowed sum
                # S = sum(a*[a<11.5]) comes from two ACT accumulations:
                #   R = sum relu(12 - a)   (= 12*N - S over a < 12)
                #   G = sum sign(a - 12)   (= M - 2N; a == 12 has measure
                #       zero since a = |60r - 60v| with continuous r)
                # ACT is immune to the gpsimd SBUF-port contention that
                # cripples DVE during local_scatter.
                e2 = lossp.tile([128, LCW], f32, tag="e2", name=f"e2_{c}")
                nc.vector.tensor_tensor(
                    out=e2[:], in0=refs_sb[c][:], in1=tbl[c][:],
                    op=Alu.subtract,
                )
                a = lossp.tile([128, LCW], f32, tag="a", name=f"a{c}")
                nc.scalar.activation(a[:], e2[:], Act.Abs)
                r1 = lossp.tile([128, LCW], f32, tag="r1", name=f"r1_{c}")
                nc.scalar.activation(
                    r1[:], a[:], Act.Relu, bias=bcol[:, 0:1], scale=-1.0,
                    accum_out=rparts[c][:],
                )
                sg = lossp.tile([128, LCW], f32, tag="sg", name=f"sg{c}")
                nc.scalar.activation(
                    sg[:], a[:], Act.Sign, bias=bcol[:, 1:2], scale=1.0,
                    accum_out=gparts[c][:],
                )
                nc.scalar.dma_start(out[:, 2 * c:2 * c + 1], rparts[c][:])
                nc.scalar.dma_start(out[:, 2 * c + 1:2 * c + 2], gparts[c][:])

            # pair 0 is processed as two single-k halves so the first
            # scatter launches ~5us earlier (halved critical chain)
            for half0 in range(2):
                sl = slice(half0 * 512, half0 * 512 + 512)
                if half0 == 0:
                    g0h = workp.tile([128, 512], f16, tag="g", name="g0a")
                    nc.scalar.activation(g0h[:], uts[0][:, sl], Act.Copy,
                                         bias=1044.0, scale=50.0)
                else:
                    g0h = workp.tile([128, 512], f16, tag="g", name="g0b")
                    nc.scalar.activation(g0h[:], uts[0][:, sl], Act.Copy,
                                         bias=1044.0, scale=50.0)
                mkh = workp.tile([128, 512], f16, tag="mk", name=f"mkh{half0}")
                nc.vector.tensor_scalar(
                    mkh[:], uts[0][:, sl], 0.0235, MASK_SUB,
                    op0=Alu.is_lt, op1=Alu.mult
                )
                mkxh = workp.tile([128, 512], f16, tag="mkx",
                                  name=f"mkxh{half0}")
                nc.vector.tensor_tensor(
                    out=mkxh[:], in0=mkh[:], in1=cofs[:, 0:512],
                    op=Alu.subtract
                )
                ixh = workp.tile([128, 512], f16, tag="ix", name=f"ixh{half0}")
                nc.vector.tensor_tensor(
                    out=ixh[:], in0=g0h[:], in1=mkxh[:], op=Alu.subtract
                )
                psh = psump.tile([128, 512], f16, tag="ps", name=f"psh{half0}",
                                 space="PSUM")
                nc.tensor.transpose(psh[:, 0:128], ixh[:, 0:128], antid[:])
                nc.tensor.transpose(psh[:, 128:256], ixh[:, 256:384],
                                    ident[:])
                nc.tensor.transpose(psh[:, 256:384], ixh[:, 128:256],
                                    antid[:])
                nc.tensor.transpose(psh[:, 384:512], ixh[:, 384:512],
                                    ident[:])
                sth = workp.tile([128, 512], i16, tag="st", name=f"sth{half0}")
                nc.scalar.activation(sth[:], psh[:], Act.Copy, bias=-1045.0,
                                     scale=1.0)
                nc.gpsimd.local_scatter(
                    tbl[0][:, half0 * KTBL:(half0 + 1) * KTBL],
                    vee[:, 0:512],
                    sth[:],
                    channels=128,
                    num_elems=KTBL,
                    num_idxs=512,
                )
            uts.pop(0)
            emit_g(1)
            fetch_ut(3)
            fetch_refs(1)

            for p in range(1, NPAIR):
                if p + 3 < NPAIR:
                    fetch_ut(p + 3)
                if p % 2 == 0 and p // 2 + 1 < NCHUNK:
                    fetch_refs(p // 2 + 1)
                if p + 1 < NPAIR:
                    emit_g(p + 1)

                ut = uts.pop(p)
                g = gs.pop(p)

                # mkx = (u < 0.0235)*4000 - cofs; ix = g - mkx
                mk = workp.tile([128, 1024], f16, tag="mk")
                nc.vector.tensor_scalar(
                    mk[:], ut[:], 0.0235, MASK_SUB, op0=Alu.is_lt, op1=Alu.mult
                )
                mkx = workp.tile([128, 1024], f16, tag="mkx")
                nc.vector.tensor_tensor(
                    out=mkx[:], in0=mk[:], in1=cofs[:], op=Alu.subtract
                )
                ix = workp.tile([128, 1024], f16, tag="ix")
                nc.vector.tensor_tensor(
                    out=ix[:], in0=g[:], in1=mkx[:], op=Alu.subtract
                )

                # transpose to partition=j; 8 blocks (k-parity, jt, h)
                ps = psump.tile([128, 1024], f16, tag="ps", space="PSUM")
                for kk in range(2):
                    o = kk * 512
                    nc.tensor.transpose(ps[:, o:o + 128],
                                        ix[:, o:o + 128], antid[:])
                    nc.tensor.transpose(ps[:, o + 128:o + 256],
                                        ix[:, o + 256:o + 384], ident[:])
                    nc.tensor.transpose(ps[:, o + 256:o + 384],
                                        ix[:, o + 128:o + 256], antid[:])
                    nc.tensor.transpose(ps[:, o + 384:o + 512],
                                        ix[:, o + 384:o + 512], ident[:])

                # idx = ps - 1045 in [0,400) valid, negative = skip
                st = workp.tile([128, 1024], i16, tag="st")
                nc.scalar.activation(st[:], ps[:], Act.Copy, bias=-1045.0,
                                     scale=1.0)

                c, half = p // 2, p % 2
                nc.gpsimd.local_scatter(
                    tbl[c][:, half * CHW:(half + 1) * CHW],
                    vee[:],
                    st[:],
                    channels=128,
                    num_elems=2 * KTBL,
                    num_idxs=1024,
                )
                if p >= 4 and p % 2 == 0:
                    emit_loss((p - 4) // 2)

            for c in range(NCHUNK - 2, NCHUNK):
                emit_loss(c)

    nc.compile()
    return nc


def _host_constants():
    # scatter data stream values per 256-block: [left: n+1 | right: n-128],
    # +OFF; garbage 25000 at the right-stream head (i==128, value 0 in the
    # reference -> must never produce a live table entry by itself)
    n = np.arange(256)
    blk = np.where(n < 128, n + 1, n - 128).astype(np.int64) + int(OFF)
    blk[128] = 25000
    vee = np.tile(blk, 4).astype(np.int16)
    vee = np.ascontiguousarray(np.broadcast_to(vee, (128, 1024)))

    # cofs[col]: block col offset {0,50,100,150} + 200*(k%2);
    # pre-transpose layout: col = (k%2)*512 + h*256 + j
    col = np.arange(1024)
    kk = col // 512
    h = (col % 512) // 256
    j = col % 256
    jt = (j // 128).astype(np.int64)
    cofs = (200 * kk + 100 * jt + 50 * h).astype(np.float32)
    cofs = np.ascontiguousarray(
        np.broadcast_to(cofs.astype(np.float16), (128, 1024))
    )

    ident = np.eye(128, dtype=np.float32).astype(np.float16)
    antid = np.ascontiguousarray(ident[::-1, :])
    return vee, cofs, ident, antid


def _prep_refs(left, right):
    """[CHUNKS*128, CHW] per core; table col (within core) =
    k*200 + block*50 + (bin-111), block in [jt0L, jt0R, jt1L, jt1R],
    channel = j mod 128; values pre-scaled: 60*ref + OFF."""
    lft = left[:, 0, :, 111:161]    # [B, W, 50]
    rgt = right[:, 0, :, 111:161]
    refs = np.empty((NCORES, KPC, 4, 128, NBIN), np.float32)
    for core in range(NCORES):
        for k in range(KPC):
            kg = core * KPC + k
            refs[core, k, 0] = lft[kg, 0:128, :]
            refs[core, k, 1] = rgt[kg, 0:128, :]
            refs[core, k, 2] = lft[kg, 128:256, :]
            refs[core, k, 3] = rgt[kg, 128:256, :]
    refs = refs * 60.0 + np.float32(OFF)
    # -> [core, chunk, channel, (k_in_chunk, block, bin)]
    refs = refs.reshape(NCORES, NCHUNK, 4, 4, 128, NBIN)
    refs = refs.transpose(0, 1, 4, 2, 3, 5)
    return np.ascontiguousarray(
        refs.reshape(NCORES, NCHUNK * 128, LCW).astype(np.float32)
    )


def make_in_maps(up, left, right):
    up = np.asarray(up, np.float32)
    left = np.asarray(left, np.float32)
    right = np.asarray(right, np.float32)
    vee, cofs, ident, antid = _host_constants()
    bcol = np.ascontiguousarray(np.broadcast_to(
        np.array([12.0, -12.0], np.float32), (128, 2)))
    refs = _prep_refs(left, right)
    in_maps = []
    for c in range(NCORES):
        upc = np.ascontiguousarray(
            up[c * KPC:(c + 1) * KPC, 0].reshape(KPC * H, W)
        )
        in_maps.append({
            "up_in": upc,
            "refs_in": refs[c],
            "bcol_in": bcol,
            "vee_in": vee,
            "cofs_in": cofs,
            "ident_in": ident,
            "antid_in": antid,
        })
    return in_maps


def get_nc():
    if "nc" not in _cache:
        _cache["nc"] = _build_bass()
    return _cache["nc"]


def reduce_results(results):
    # out cols: [R_0, G_0, R_1, G_1, ...] per chunk;
    # S = sum a*[a<12] = 12*N - R with N = (M - G)/2, M = LCW per
    # partition per chunk
    total = 0.0
    for r in results:
        o = np.asarray(r["out"]).astype(np.float64)
        R = o[:, 0::2].sum()
        G = o[:, 1::2].sum()
        M = 128.0 * LCW * NCHUNK
        N = (M - G) / 2.0
        total += 12.0 * N - R
    return np.float32(total / (60.0 * B * W * W))


def kernel(up, left, right):
    nc = get_nc()
    in_maps = make_in_maps(up, left, right)
    res = run_bass_kernel_spmd(nc, in_maps, core_ids=list(range(NCORES)))
    return reduce_results(res.results)
